# revision 22
# baseline (speedup 1.0000x reference)
"""Llama4VisionAttention on 8 Trainium2 NeuronCores.

Data-parallel over batch (32 images -> 4 per core), weights distributed by
in-NEFF AllGather so each core only uploads 1/8 of the weight pack.

Host -> device traffic per core (all fp16):
  xin  [2308, 1408]  token-major hidden states (4 images), no padding
  ws   [1, 1081344]  this core's 1/8 shard of the packed augmented weights
  cos/sinp [88, 578] rope tables
Device -> host: out [2308, 1408] fp16.

Kernel phases:
  W : stage ws to internal DRAM, AllGather -> wfull [4, 1536, 1408]
      (wq_a, wk_a, wv_a, wo_a; row 1408 = bias, rows 1409.. = 0).
  A0: PE-transpose xin into SBUF xt_sb [128, 12, TOKP] feature-major,
      k-tile 11 holds the bias ones-row.
  A : QKV projections (fp16 matmuls, fp32 PSUM accum), Q/K spilled
      feature-major, V token-major, all fp16 DRAM scratch.
  B : per (image, head): reload Q/K head-major [88, SP] (+ pair-swapped
      copy for ROPE), ROPE on DVE, scores on PE, exp on ACT, A@V with a
      ones-column for the softmax denominator, PE ones-broadcast of
      1/rowsum, normalize, spill context feature-major.
  D : O-projection with fused bias (ones-row trick), token-major fp16 out.

Execution: cached jax.jit(shard_map(bass_exec)) over 8 cores; donated
zero output buffers are created on-device (never uploaded).
"""

import math

import numpy as np
import jax
import jax.numpy as jnp
from jax.sharding import Mesh, PartitionSpec, NamedSharding
from jax.experimental.shard_map import shard_map

import concourse.bass as bass
import concourse.tile as tile
from concourse import bacc, mybir
from concourse.bass2jax import _bass_exec_p, install_neuronx_cc_hook
from concourse.masks import make_identity

F32 = mybir.dt.float32
F32R = mybir.dt.float32r
FP16 = mybir.dt.float16

H = 16
HD = 88
D = 1408          # = H * HD
S = 577
SP = 578          # padded (even) seq
B = 32
NCORES = 8
BPC = B // NCORES  # images per core
TOK = BPC * S      # 2308
TOKP = 2376        # padded xt free dim (>= 3*577 + 512 + 128 = 2371)
KP = 1536          # padded contraction dim (12 * 128)
NKT = KP // 128
CPAD = 640         # padded ctx columns (5 * 128)
WSH = 4 * KP * D // NCORES  # weight-pack shard elems per core
SCALE = 1.0 / math.sqrt(HD)

QCH = ((0, 290), (290, 287))
QCH2 = ((0, 512), (512, 512), (1024, 512), (1536, 512), (2048, 260))
VCH = ((0, 512), (512, 512), (1024, 384))
KTS = ((0, 128), (128, 128), (256, 128), (384, 128), (512, 65))
TTS = tuple((t0, min(128, TOK - t0)) for t0 in range(0, TOK, 128))


def _rope_tables():
    IDX = 24
    theta = 10000.0
    img_idx = np.arange(IDX * IDX, dtype=np.int64).reshape(-1, 1)
    img_idx = np.concatenate([img_idx, img_idx[:1]], axis=0)
    img_idx[-1, -1] = -2
    fx = (img_idx % IDX).astype(np.float64)
    fy = (img_idx // IDX).astype(np.float64)
    freq_dim = HD // 2
    rope_freq = 1.0 / (theta ** (np.arange(0, freq_dim, 2)[: freq_dim // 2].astype(np.float64) / freq_dim))
    fxf = (fx + 1)[..., None] * rope_freq[None, None, :]
    fyf = (fy + 1)[..., None] * rope_freq[None, None, :]
    freqs = np.concatenate([fxf[..., 0, :], fyf[..., 0, :]], axis=-1)  # [S,44]
    freqs = np.where(img_idx.reshape(-1, 1) < 0, 0.0, freqs)
    cos = np.cos(freqs)
    sin = np.sin(freqs)
    cos_t = np.ones((HD, SP), np.float32)
    sinp_t = np.zeros((HD, SP), np.float32)
    for hd in range(HD):
        i = hd // 2
        cos_t[hd, :S] = cos[:, i]
        sinp_t[hd, :S] = (-sin[:, i]) if hd % 2 == 0 else sin[:, i]
    return cos_t.astype(np.float16), sinp_t.astype(np.float16)


_CACHE = {}


def _build():
    nc = bacc.Bacc("TRN2", target_bir_lowering=False, debug=False, num_devices=NCORES)

    xin_d = nc.dram_tensor("xin", [TOK, D], FP16, kind="ExternalInput").ap()
    ws_d = nc.dram_tensor("ws", [1, WSH], FP16, kind="ExternalInput").ap()
    cos_d = nc.dram_tensor("cos", [HD, SP], FP16, kind="ExternalInput").ap()
    sinp_d = nc.dram_tensor("sinp", [HD, SP], FP16, kind="ExternalInput").ap()
    out_d = nc.dram_tensor("out", [TOK, D], FP16, kind="ExternalOutput").ap()

    from contextlib import ExitStack
    with tile.TileContext(nc) as tc, nc.allow_low_precision(reason="fp16 intermediates; matmuls accumulate fp32 in PSUM"):
        with ExitStack() as ctx:
            dpool = ctx.enter_context(tc.tile_pool(name="dram", bufs=1, space="DRAM"))
            cpool = ctx.enter_context(tc.tile_pool(name="const", bufs=1))
            psA = ctx.enter_context(tc.tile_pool(name="psA", bufs=3, space="PSUM"))
            psS = ctx.enter_context(tc.tile_pool(name="psS", bufs=2, space="PSUM"))
            psC = ctx.enter_context(tc.tile_pool(name="psC", bufs=2, space="PSUM"))
            psB = ctx.enter_context(tc.tile_pool(name="psB", bufs=1, space="PSUM"))
            qkpool = ctx.enter_context(tc.tile_pool(name="qk", bufs=2))
            epool = ctx.enter_context(tc.tile_pool(name="es", bufs=7))
            vtpool = ctx.enter_context(tc.tile_pool(name="vt", bufs=10))
            ipool = ctx.enter_context(tc.tile_pool(name="inv", bufs=2))
            bpool = ctx.enter_context(tc.tile_pool(name="bb", bufs=2))
            ctxpool = ctx.enter_context(tc.tile_pool(name="ctx", bufs=3))

            qs_t = dpool.tile([D, TOK], FP16)
            ks_t = dpool.tile([D, TOK], FP16)
            vs_t = dpool.tile([BPC, S, D], FP16)
            cs_t = dpool.tile([BPC, D, CPAD], FP16)
            wsh_t = dpool.tile([1, WSH], FP16)
            wfull_t = dpool.tile([4, KP, D], FP16)
            qs_d, ks_d, vs_d, cs_d = qs_t[:], ks_t[:], vs_t[:], cs_t[:]
            wfull_d = wfull_t[:]

            cos_sb = cpool.tile([HD, SP], FP16)
            sinp_sb = cpool.tile([HD, SP], FP16)
            nc.sync.dma_start(cos_sb[:], cos_d[:])
            nc.sync.dma_start(sinp_sb[:], sinp_d[:])
            ones1 = cpool.tile([1, 128], FP16)
            nc.gpsimd.memset(ones1[:], 1.0)
            onespad = cpool.tile([128, 128], FP16)
            nc.gpsimd.memset(onespad[:], 0.0)
            nc.gpsimd.memset(onespad[0:1, :], 1.0)
            ident = cpool.tile([128, 128], FP16)
            make_identity(nc, ident[:])

            # ---------------- Phase W: weight AllGather --------------------
            nc.sync.dma_start(wsh_t[:], ws_d[:])
            nc.gpsimd.collective_compute(
                "AllGather",
                mybir.AluOpType.bypass,
                replica_groups=[list(range(NCORES))],
                ins=[wsh_t[:]],
                outs=[wfull_t[:]],
            )

            # ---------------- Phase A (transient pools) ---------------------
            ctxA = ExitStack()
            with ctxA:
                xpool = ctxA.enter_context(tc.tile_pool(name="xt", bufs=1))
                xrpool = ctxA.enter_context(tc.tile_pool(name="xr", bufs=3))
                wpool = ctxA.enter_context(tc.tile_pool(name="wqk", bufs=2))
                vwpool = ctxA.enter_context(tc.tile_pool(name="wv", bufs=1))
                stpool = ctxA.enter_context(tc.tile_pool(name="stqk", bufs=4))

                # Phase A0: transpose xin -> xt_sb feature-major
                xt_sb = xpool.tile([128, NKT, TOKP], FP16)
                nc.vector.memset(xt_sb[:, 11, :], 0.0)
                nc.vector.memset(xt_sb[0:1, 11, :TOK], 1.0)
                for kt in range(11):
                    nc.vector.memset(xt_sb[:, kt, TOK:TOKP], 0.0)
                for (t0, rows) in TTS:
                    xr = xrpool.tile([128, D], FP16, tag="xr")
                    nc.sync.dma_start(xr[:rows, :], xin_d[t0:t0 + rows, :])
                    for ft in range(11):
                        pt = psS.tile([128, 128], FP16, tag="psS")
                        nc.tensor.transpose(
                            pt[:, :rows], xr[:rows, ft * 128:(ft + 1) * 128],
                            ident[:rows, :rows])
                        nc.scalar.copy(xt_sb[:, ft, t0:t0 + rows], pt[:, :rows])

                # [128,1] bias columns for Q/K: load the bias rows
                # contiguously, then PE-transpose 128-chunks (a scatter DMA
                # per column would cost ~10us of descriptor issue each)
                biasqk = cpool.tile([128, 2, 11], FP16)
                brow_q = cpool.tile([1, D], FP16)
                brow_k = cpool.tile([1, D], FP16)
                brows = [brow_q, brow_k]
                for w_idx in (0, 1):
                    nc.sync.dma_start(brows[w_idx][:], wfull_d[w_idx, D:D + 1, :])
                for w_idx in (0, 1):
                    for dt_ in range(11):
                        pt = psS.tile([128, 128], FP16, tag="psS")
                        nc.tensor.transpose(
                            pt[:, 0:1],
                            brows[w_idx][:, dt_ * 128:(dt_ + 1) * 128],
                            ident[:1, :1])
                        nc.scalar.copy(biasqk[:, w_idx, dt_:dt_ + 1], pt[:, 0:1])

                # Q/K projections, feature-major spill, 512-wide chains
                # across image boundaries (flat [D, TOK] scratch layout).
                # Bias is fused into the PSUM->SBUF copy (per-partition ACT
                # bias), so the chains skip the 12th (bias ones-row) k-tile.
                for w_idx, sp_d in ((0, qs_d), (1, ks_d)):
                    for dt_ in range(11):
                        wt = wpool.tile([128, NKT - 1, 128], FP16, tag="wqk")
                        w_r = wfull_d[w_idx, :D, dt_ * 128:(dt_ + 1) * 128].rearrange(
                            "(kt p) d -> p kt d", p=128)
                        nc.gpsimd.dma_start(wt[:], w_r)
                        for (c0, cw) in QCH2:
                            ps = psA.tile([128, 512], F32, tag="psA")
                            for kt in range(NKT - 1):
                                nc.tensor.matmul(
                                    ps[:, :cw],
                                    wt[:, kt, :],
                                    xt_sb[:, kt, c0:c0 + cw],
                                    start=(kt == 0), stop=(kt == NKT - 2),
                                )
                            st = stpool.tile([128, 512], FP16, tag="stv")
                            nc.scalar.activation(
                                st[:, :cw], ps[:, :cw],
                                mybir.ActivationFunctionType.Identity,
                                bias=biasqk[:, w_idx, dt_:dt_ + 1],
                            )
                            nc.scalar.dma_start(
                                sp_d[dt_ * 128:(dt_ + 1) * 128, c0:c0 + cw],
                                st[:, :cw],
                            )

                # V projection, token-major spill
                for (c0, cw) in VCH:
                    vw = vwpool.tile([128, NKT, 512], FP16, tag="wv")
                    w_r = wfull_d[2, :, c0:c0 + cw].rearrange("(kt p) d -> p kt d", p=128)
                    nc.gpsimd.dma_start(vw[:, :, :cw], w_r)
                    for img in range(BPC):
                        for tt in range(5):
                            t0 = img * S + tt * 128
                            ps = psA.tile([128, 512], F32, tag="psA")
                            for kt in range(NKT):
                                nc.tensor.matmul(
                                    ps[:, :cw],
                                    xt_sb[:, kt, t0:t0 + 128],
                                    vw[:, kt, :cw],
                                    start=(kt == 0), stop=(kt == NKT - 1),
                                )
                            rows = 65 if tt == 4 else 128
                            st = stpool.tile([128, 512], FP16, tag="stv")
                            nc.scalar.copy(st[:rows, :cw], ps[:rows, :cw])
                            nc.sync.dma_start(
                                vs_d[img, tt * 128:tt * 128 + rows, c0:c0 + cw],
                                st[:rows, :cw],
                            )

            # ---------------- Phase B: attention ---------------------------
            qs_sw = qs_d.rearrange("(p two) t -> p two t", two=2)
            ks_sw = ks_d.rearrange("(p two) t -> p two t", two=2)
            for img in range(BPC):
                tb = img * S
                for h in range(H):
                    r0 = h * HD
                    hp = r0 // 2
                    tiles = {}
                    for nm, src, swsrc in (("q", qs_d, qs_sw), ("k", ks_d, ks_sw)):
                        # kh is 640 wide so every score-stationary slice is a
                        # full 128 columns (FWL); cols S:640 are garbage and
                        # only feed PSUM rows >= ksz, which are never read
                        kw = 640 if nm == "k" else S
                        t_ = qkpool.tile([HD, kw], FP16, tag=nm)
                        nc.sync.dma_start(t_[:, :S], src[r0:r0 + HD, tb:tb + S])
                        tsw = qkpool.tile([HD, S], FP16, tag=nm + "s")
                        tsw_r = tsw[:].rearrange("(p two) t -> p two t", two=2)
                        # issue on ACT queue: SP.SEQ is the DMA-issue bottleneck
                        nc.scalar.dma_start(tsw_r[:, 0, :], swsrc[hp:hp + 44, 1, tb:tb + S])
                        nc.scalar.dma_start(tsw_r[:, 1, :], swsrc[hp:hp + 44, 0, tb:tb + S])
                        nc.vector.tensor_mul(t_[:, :S], t_[:, :S], cos_sb[:, :S])
                        nc.vector.tensor_mul(tsw[:], tsw[:], sinp_sb[:, :S])
                        nc.vector.tensor_add(t_[:, :S], t_[:, :S], tsw[:])
                        tiles[nm] = t_
                    qh, kh = tiles["q"], tiles["k"]
                    nc.vector.memset(kh[:, S:640], 0.0)

                    es = []
                    for (k0, ksz) in KTS:
                        e_ = epool.tile([128, S], FP16, tag="es")
                        for (c0, cw) in QCH:
                            ps = psS.tile([128, 290], F32, tag="psS")
                            nc.tensor.matmul(
                                ps[:, :cw], kh[:, k0:k0 + 128],
                                qh[:, c0:c0 + cw], start=True, stop=True,
                            )
                            nc.scalar.activation(
                                e_[:ksz, c0:c0 + cw], ps[:ksz, :cw],
                                mybir.ActivationFunctionType.Exp, scale=SCALE,
                            )
                        es.append(e_)

                    vts = []
                    for (k0, ksz) in KTS:
                        # 128-wide stationary (not 97): FWL only triggers for
                        # exactly-128-column weight loads on real HW
                        vt = vtpool.tile([128, 128], FP16, tag="vt")
                        nc.sync.dma_start(
                            vt[:ksz, :HD], vs_d[img, k0:k0 + ksz, r0:r0 + HD])
                        nc.vector.memset(vt[:ksz, HD:96], 0.0)
                        nc.vector.memset(vt[:ksz, 96:97], 1.0)
                        nc.vector.memset(vt[:ksz, 97:128], 0.0)
                        vts.append(vt)

                    for (c0, cw) in QCH:
                        # rows 97:128 of pc are never read
                        pc = psC.tile([128, 290], F32, tag="psC")
                        for j, (k0, ksz) in enumerate(KTS):
                            nc.tensor.matmul(
                                pc[:, :cw], vts[j][:ksz, :],
                                es[j][:ksz, c0:c0 + cw],
                                start=(j == 0), stop=(j == len(KTS) - 1),
                            )
                        inv = ipool.tile([1, 290], FP16, tag="inv")
                        nc.vector.reciprocal(inv[:, :cw], pc[96:97, :cw])
                        pb = psB.tile([128, 290], F32, tag="psB")
                        nc.tensor.matmul(pb[:, :cw], ones1[:], inv[:, :cw],
                                         start=True, stop=True)
                        bb = bpool.tile([HD, 290], F32, tag="bb")
                        nc.scalar.copy(bb[:, :cw], pb[:HD, :cw])
                        stg = ctxpool.tile([HD, 290], FP16, tag="ctx")
                        nc.vector.tensor_mul(stg[:, :cw], pc[:HD, :cw], bb[:, :cw])
                        sw = min(cw, S - c0)
                        nc.sync.dma_start(
                            cs_d[img, r0:r0 + HD, c0:c0 + sw], stg[:, :sw])

            # ---------------- Phase D: O projection -------------------------
            wopool = ctx.enter_context(tc.tile_pool(name="wo", bufs=1))
            c2pool = ctx.enter_context(tc.tile_pool(name="ct2", bufs=13))
            opool = ctx.enter_context(tc.tile_pool(name="outp", bufs=3))
            wos = []
            for (c0, cw) in VCH:
                wo_t = wopool.tile([128, NKT, cw], FP16, tag=f"wo{c0}")
                w_r = wfull_d[3, :, c0:c0 + cw].rearrange("(kt p) d -> p kt d", p=128)
                nc.gpsimd.dma_start(wo_t[:], w_r)
                wos.append(wo_t)
            for img in range(BPC):
                # batched context load: 11 wide DMAs per image on the Pool
                # queue instead of 55 x [128,128] on SP
                cts = []
                for kt in range(11):
                    ct = c2pool.tile([128, CPAD], FP16, tag="ct2")
                    nc.sync.dma_start(ct[:], cs_d[img, kt * 128:(kt + 1) * 128, :])
                    cts.append(ct)
                for tt in range(5):
                    rows = 65 if tt == 4 else 128
                    for ci, (c0, cw) in enumerate(VCH):
                        ps = psA.tile([128, 512], F32, tag="psA")
                        for kt in range(11):
                            nc.tensor.matmul(
                                ps[:, :cw],
                                cts[kt][:, tt * 128:(tt + 1) * 128],
                                wos[ci][:, kt, :],
                                start=(kt == 0), stop=False,
                            )
                        nc.tensor.matmul(
                            ps[:, :cw], onespad[:], wos[ci][:, 11, :],
                            start=False, stop=True,
                        )
                        ot = opool.tile([128, 512], FP16, tag="outp")
                        nc.scalar.copy(ot[:rows, :cw], ps[:rows, :cw])
                        nc.sync.dma_start(
                            out_d[img * S + tt * 128: img * S + tt * 128 + rows,
                                  c0:c0 + cw],
                            ot[:rows, :cw],
                        )

    nc.compile()
    return nc


def _setup():
    nc = _build()
    install_neuronx_cc_hook()

    partition_name = nc.partition_id_tensor.name if nc.partition_id_tensor else None
    in_names, out_names, out_avals = [], [], []
    for alloc in nc.m.functions[0].allocations:
        if not isinstance(alloc, mybir.MemoryLocationSet):
            continue
        name = alloc.memorylocations[0].name
        if alloc.kind == "ExternalInput":
            if name != partition_name:
                in_names.append(name)
        elif alloc.kind == "ExternalOutput":
            out_names.append(name)
            out_avals.append(jax.core.ShapedArray(
                tuple(alloc.tensor_shape), mybir.dt.np(alloc.dtype)))
    n_params = len(in_names)
    n_outs = len(out_avals)
    in_names_all = in_names + out_names
    if partition_name is not None:
        in_names_all.append(partition_name)

    def _body(*args):
        operands = list(args)
        if partition_name is not None:
            from concourse.bass2jax import partition_id_tensor
            operands.append(partition_id_tensor())
        outs = _bass_exec_p.bind(
            *operands,
            out_avals=tuple(out_avals),
            in_names=tuple(in_names_all),
            out_names=tuple(out_names),
            lowering_input_output_aliases=(),
            sim_require_finite=True,
            sim_require_nnan=True,
            nc=nc,
        )
        return tuple(outs)

    devices = jax.devices()[:NCORES]
    mesh = Mesh(np.asarray(devices), ("core",))
    in_specs = (PartitionSpec("core"),) * (n_params + n_outs)
    out_specs = (PartitionSpec("core"),) * n_outs
    donate = tuple(range(n_params, n_params + n_outs))
    exec_fn = jax.jit(
        shard_map(_body, mesh=mesh, in_specs=in_specs, out_specs=out_specs,
                  check_rep=False),
        donate_argnums=donate, keep_unused=True)

    shard = NamedSharding(mesh, PartitionSpec("core"))
    zero_shapes = [(NCORES * a.shape[0], *a.shape[1:]) for a in out_avals]
    zero_dtypes = [a.dtype for a in out_avals]
    make_zeros = jax.jit(
        lambda: tuple(jnp.zeros(s, d) for s, d in zip(zero_shapes, zero_dtypes)),
        out_shardings=tuple([shard] * n_outs))

    return dict(nc=nc, in_names=in_names, exec_fn=exec_fn,
                make_zeros=make_zeros, shard=shard, mesh=mesh)


def _host_pack(hidden_states, wq, bq, wk, bk, wv, bv, wo, bo, cos_t, sinp_t):
    """Build the global (concatenated-over-cores) input arrays, all fp16."""
    x16 = np.asarray(hidden_states, np.float32).reshape(NCORES * TOK, D).astype(np.float16)
    wp = np.zeros((4, KP, D), np.float16)
    for i, (w, b) in enumerate(((wq, bq), (wk, bk), (wv, bv), (wo, bo))):
        wp[i, :D] = np.asarray(w, np.float32).T.astype(np.float16)
        wp[i, D] = np.asarray(b, np.float32).astype(np.float16)
    ws = wp.reshape(NCORES, WSH)
    return {
        "xin": x16,
        "ws": ws,
        "cos": np.tile(cos_t, (NCORES, 1)),
        "sinp": np.tile(sinp_t, (NCORES, 1)),
    }


def kernel(hidden_states, wq, bq, wk, bk, wv, bv, wo, bo):
    if "st" not in _CACHE:
        _CACHE["st"] = _setup()
        _CACHE["tables"] = _rope_tables()
    st = _CACHE["st"]
    cos_t, sinp_t = _CACHE["tables"]

    # donated output buffers are device-created; issue the dispatch first so
    # it rides the tunnel while the host packs/uploads the inputs
    zeros = st["make_zeros"]()
    arrs = _host_pack(hidden_states, wq, bq, wk, bk, wv, bv, wo, bo, cos_t, sinp_t)
    args = [arrs[n] for n in st["in_names"]]
    try:
        outs = st["exec_fn"](*args, *zeros)
        out = np.asarray(outs[0])
    except jax.errors.JaxRuntimeError:
        # transient tunnel/dispatch failure: retry once with fresh buffers
        zeros = st["make_zeros"]()
        outs = st["exec_fn"](*args, *zeros)
        out = np.asarray(outs[0])
    return out.astype(np.float32).reshape(B, S, D)


# revision 24
# speedup vs baseline: 1.1160x; 1.1160x over previous
"""Llama4VisionAttention on 8 Trainium2 NeuronCores.

Data-parallel over batch (32 images -> 4 per core), weights distributed by
in-NEFF AllGather so each core only uploads 1/8 of the weight pack.

Host -> device traffic per core (all fp16):
  xin  [2308, 1408]  token-major hidden states (4 images), no padding
  ws   [1, 1081344]  this core's 1/8 shard of the packed augmented weights
  cos/sinp [88, 578] rope tables
Device -> host: out [2308, 1408] fp16.

Kernel phases:
  W : stage ws to internal DRAM, AllGather -> wfull [4, 1536, 1408]
      (wq_a, wk_a, wv_a, wo_a; row 1408 = bias, rows 1409.. = 0).
  A0: PE-transpose xin into SBUF xt_sb [128, 12, TOKP] feature-major,
      k-tile 11 holds the bias ones-row.
  A : QKV projections (fp16 matmuls, fp32 PSUM accum), Q/K spilled
      feature-major, V token-major, all fp16 DRAM scratch.
  B : per (image, head): reload Q/K head-major [88, SP] (+ pair-swapped
      copy for ROPE), ROPE on DVE, scores on PE, exp on ACT, A@V with a
      ones-column for the softmax denominator, PE ones-broadcast of
      1/rowsum, normalize, spill context feature-major.
  D : O-projection with fused bias (ones-row trick), token-major fp16 out.

Execution: cached jax.jit(shard_map(bass_exec)) over 8 cores; donated
zero output buffers are created on-device (never uploaded).
"""

import math

import numpy as np
import jax
import jax.numpy as jnp
from jax.sharding import Mesh, PartitionSpec, NamedSharding
from jax.experimental.shard_map import shard_map

import concourse.bass as bass
import concourse.tile as tile
from concourse import bacc, mybir
from concourse.bass2jax import _bass_exec_p, install_neuronx_cc_hook
from concourse.masks import make_identity

F32 = mybir.dt.float32
F32R = mybir.dt.float32r
FP16 = mybir.dt.float16

H = 16
HD = 88
D = 1408          # = H * HD
S = 577
SP = 578          # padded (even) seq
B = 32
NCORES = 8
BPC = B // NCORES  # images per core
TOK = BPC * S      # 2308
TOKP = 2376        # padded xt free dim (>= 3*577 + 512 + 128 = 2371)
KP = 1536          # padded contraction dim (12 * 128)
NKT = KP // 128
CPAD = 640         # padded ctx columns (5 * 128)
WSH = 4 * KP * D // NCORES  # weight-pack shard elems per core
SCALE = 1.0 / math.sqrt(HD)

QCH = ((0, 290), (290, 287))
QCH2 = ((0, 512), (512, 512), (1024, 512), (1536, 512), (2048, 260))
VCH = ((0, 512), (512, 512), (1024, 384))
KTS = ((0, 128), (128, 128), (256, 128), (384, 128), (512, 65))
TTS = tuple((t0, min(128, TOK - t0)) for t0 in range(0, TOK, 128))


def _rope_tables():
    IDX = 24
    theta = 10000.0
    img_idx = np.arange(IDX * IDX, dtype=np.int64).reshape(-1, 1)
    img_idx = np.concatenate([img_idx, img_idx[:1]], axis=0)
    img_idx[-1, -1] = -2
    fx = (img_idx % IDX).astype(np.float64)
    fy = (img_idx // IDX).astype(np.float64)
    freq_dim = HD // 2
    rope_freq = 1.0 / (theta ** (np.arange(0, freq_dim, 2)[: freq_dim // 2].astype(np.float64) / freq_dim))
    fxf = (fx + 1)[..., None] * rope_freq[None, None, :]
    fyf = (fy + 1)[..., None] * rope_freq[None, None, :]
    freqs = np.concatenate([fxf[..., 0, :], fyf[..., 0, :]], axis=-1)  # [S,44]
    freqs = np.where(img_idx.reshape(-1, 1) < 0, 0.0, freqs)
    cos = np.cos(freqs)
    sin = np.sin(freqs)
    cos_t = np.ones((HD, SP), np.float32)
    sinp_t = np.zeros((HD, SP), np.float32)
    for hd in range(HD):
        i = hd // 2
        cos_t[hd, :S] = cos[:, i]
        sinp_t[hd, :S] = (-sin[:, i]) if hd % 2 == 0 else sin[:, i]
    return cos_t.astype(np.float16), sinp_t.astype(np.float16)


_CACHE = {}


def _build():
    nc = bacc.Bacc("TRN2", target_bir_lowering=False, debug=False, num_devices=NCORES)

    xin_d = nc.dram_tensor("xin", [TOK, D], FP16, kind="ExternalInput").ap()
    ws_d = nc.dram_tensor("ws", [1, WSH], FP16, kind="ExternalInput").ap()
    cos_d = nc.dram_tensor("cos", [HD, SP], FP16, kind="ExternalInput").ap()
    sinp_d = nc.dram_tensor("sinp", [HD, SP], FP16, kind="ExternalInput").ap()
    out_d = nc.dram_tensor("out", [TOK, D], FP16, kind="ExternalOutput").ap()

    from contextlib import ExitStack
    with tile.TileContext(nc) as tc, nc.allow_low_precision(reason="fp16 intermediates; matmuls accumulate fp32 in PSUM"):
        with ExitStack() as ctx:
            dpool = ctx.enter_context(tc.tile_pool(name="dram", bufs=1, space="DRAM"))
            cpool = ctx.enter_context(tc.tile_pool(name="const", bufs=1))
            psA = ctx.enter_context(tc.tile_pool(name="psA", bufs=3, space="PSUM"))
            psS = ctx.enter_context(tc.tile_pool(name="psS", bufs=2, space="PSUM"))
            psC = ctx.enter_context(tc.tile_pool(name="psC", bufs=2, space="PSUM"))
            psB = ctx.enter_context(tc.tile_pool(name="psB", bufs=1, space="PSUM"))
            qkpool = ctx.enter_context(tc.tile_pool(name="qk", bufs=2))
            epool = ctx.enter_context(tc.tile_pool(name="es", bufs=7))
            vtpool = ctx.enter_context(tc.tile_pool(name="vt", bufs=10))
            ipool = ctx.enter_context(tc.tile_pool(name="inv", bufs=2))
            bpool = ctx.enter_context(tc.tile_pool(name="bb", bufs=2))
            ctxpool = ctx.enter_context(tc.tile_pool(name="ctx", bufs=3))

            qs_t = dpool.tile([D, TOK], FP16)
            ks_t = dpool.tile([D, TOK], FP16)
            vs_t = dpool.tile([BPC, S, D], FP16)
            cs_t = dpool.tile([BPC, D, CPAD], FP16)
            wsh_t = dpool.tile([1, WSH], FP16)
            wfull_t = dpool.tile([4, KP, D], FP16)
            qs_d, ks_d, vs_d, cs_d = qs_t[:], ks_t[:], vs_t[:], cs_t[:]
            wfull_d = wfull_t[:]

            cos_sb = cpool.tile([HD, SP], FP16)
            sinp_sb = cpool.tile([HD, SP], FP16)
            nc.sync.dma_start(cos_sb[:], cos_d[:])
            nc.sync.dma_start(sinp_sb[:], sinp_d[:])
            ones1 = cpool.tile([1, 128], FP16)
            nc.gpsimd.memset(ones1[:], 1.0)
            ident = cpool.tile([128, 128], FP16)
            make_identity(nc, ident[:])

            # ---------------- Phase W: weight AllGather --------------------
            nc.sync.dma_start(wsh_t[:], ws_d[:])
            nc.gpsimd.collective_compute(
                "AllGather",
                mybir.AluOpType.bypass,
                replica_groups=[list(range(NCORES))],
                ins=[wsh_t[:]],
                outs=[wfull_t[:]],
            )

            # ---------------- Phase A (transient pools) ---------------------
            ctxA = ExitStack()
            with ctxA:
                xpool = ctxA.enter_context(tc.tile_pool(name="xt", bufs=1))
                xrpool = ctxA.enter_context(tc.tile_pool(name="xr", bufs=3))
                wpool = ctxA.enter_context(tc.tile_pool(name="wqk", bufs=2))
                vwpool = ctxA.enter_context(tc.tile_pool(name="wv", bufs=1))
                stpool = ctxA.enter_context(tc.tile_pool(name="stqk", bufs=4))

                # Phase A0: transpose xin -> xt_sb feature-major
                xt_sb = xpool.tile([128, NKT, TOKP], FP16)
                nc.vector.memset(xt_sb[:, 11, :], 0.0)
                nc.vector.memset(xt_sb[0:1, 11, :TOK], 1.0)
                for kt in range(11):
                    nc.vector.memset(xt_sb[:, kt, TOK:TOKP], 0.0)
                for (t0, rows) in TTS:
                    xr = xrpool.tile([128, D], FP16, tag="xr")
                    nc.sync.dma_start(xr[:rows, :], xin_d[t0:t0 + rows, :])
                    for ft in range(11):
                        pt = psS.tile([128, 128], FP16, tag="psS")
                        nc.tensor.transpose(
                            pt[:, :rows], xr[:rows, ft * 128:(ft + 1) * 128],
                            ident[:rows, :rows])
                        nc.scalar.copy(xt_sb[:, ft, t0:t0 + rows], pt[:, :rows])

                # [128,1] bias columns for Q/K: load the bias rows
                # contiguously, then PE-transpose 128-chunks (a scatter DMA
                # per column would cost ~10us of descriptor issue each)
                biasqk = cpool.tile([128, 2, 11], FP16)
                brow_q = cpool.tile([1, D], FP16)
                brow_k = cpool.tile([1, D], FP16)
                brows = [brow_q, brow_k]
                for w_idx in (0, 1):
                    nc.sync.dma_start(brows[w_idx][:], wfull_d[w_idx, D:D + 1, :])
                for w_idx in (0, 1):
                    for dt_ in range(11):
                        pt = psS.tile([128, 128], FP16, tag="psS")
                        nc.tensor.transpose(
                            pt[:, 0:1],
                            brows[w_idx][:, dt_ * 128:(dt_ + 1) * 128],
                            ident[:1, :1])
                        nc.scalar.copy(biasqk[:, w_idx, dt_:dt_ + 1], pt[:, 0:1])

                # Q/K projections, feature-major spill, 512-wide chains
                # across image boundaries (flat [D, TOK] scratch layout).
                # Bias is fused into the PSUM->SBUF copy (per-partition ACT
                # bias), so the chains skip the 12th (bias ones-row) k-tile.
                for w_idx, sp_d in ((0, qs_d), (1, ks_d)):
                    for dt_ in range(11):
                        wt = wpool.tile([128, NKT - 1, 128], FP16, tag="wqk")
                        w_r = wfull_d[w_idx, :D, dt_ * 128:(dt_ + 1) * 128].rearrange(
                            "(kt p) d -> p kt d", p=128)
                        nc.gpsimd.dma_start(wt[:], w_r)
                        for (c0, cw) in QCH2:
                            ps = psA.tile([128, 512], F32, tag="psA")
                            for kt in range(NKT - 1):
                                nc.tensor.matmul(
                                    ps[:, :cw],
                                    wt[:, kt, :],
                                    xt_sb[:, kt, c0:c0 + cw],
                                    start=(kt == 0), stop=(kt == NKT - 2),
                                )
                            st = stpool.tile([128, 512], FP16, tag="stv")
                            nc.scalar.activation(
                                st[:, :cw], ps[:, :cw],
                                mybir.ActivationFunctionType.Identity,
                                bias=biasqk[:, w_idx, dt_:dt_ + 1],
                            )
                            nc.scalar.dma_start(
                                sp_d[dt_ * 128:(dt_ + 1) * 128, c0:c0 + cw],
                                st[:, :cw],
                            )

                # V projection, token-major spill.  Bias varies along the
                # free (channel) dim, so it is added as a PE-broadcast tile
                # via the DVE copy -- the chains skip the ones-row k-tile.
                brow_v = cpool.tile([1, D], FP16)
                nc.sync.dma_start(brow_v[:], wfull_d[2, D:D + 1, :])
                vbias = cpool.tile([128, D], F32)
                for (c0, cw) in VCH:
                    pb = psB.tile([128, 512], F32, tag="psB")
                    nc.tensor.matmul(pb[:, :cw], ones1[:], brow_v[:, c0:c0 + cw],
                                     start=True, stop=True)
                    nc.scalar.copy(vbias[:, c0:c0 + cw], pb[:, :cw])
                for (c0, cw) in VCH:
                    vw = vwpool.tile([128, NKT - 1, 512], FP16, tag="wv")
                    w_r = wfull_d[2, :D, c0:c0 + cw].rearrange("(kt p) d -> p kt d", p=128)
                    nc.gpsimd.dma_start(vw[:, :, :cw], w_r)
                    for img in range(BPC):
                        for tt in range(5):
                            t0 = img * S + tt * 128
                            ps = psA.tile([128, 512], F32, tag="psA")
                            for kt in range(NKT - 1):
                                nc.tensor.matmul(
                                    ps[:, :cw],
                                    xt_sb[:, kt, t0:t0 + 128],
                                    vw[:, kt, :cw],
                                    start=(kt == 0), stop=(kt == NKT - 2),
                                )
                            rows = 65 if tt == 4 else 128
                            st = stpool.tile([128, 512], FP16, tag="stv")
                            nc.vector.tensor_add(st[:rows, :cw], ps[:rows, :cw],
                                                 vbias[:rows, c0:c0 + cw])
                            nc.sync.dma_start(
                                vs_d[img, tt * 128:tt * 128 + rows, c0:c0 + cw],
                                st[:rows, :cw],
                            )

            # ---------------- Phase B: attention ---------------------------
            qs_sw = qs_d.rearrange("(p two) t -> p two t", two=2)
            ks_sw = ks_d.rearrange("(p two) t -> p two t", two=2)
            for img in range(BPC):
                tb = img * S
                for h in range(H):
                    r0 = h * HD
                    hp = r0 // 2
                    tiles = {}
                    for nm, src, swsrc in (("q", qs_d, qs_sw), ("k", ks_d, ks_sw)):
                        # kh is 640 wide so every score-stationary slice is a
                        # full 128 columns (FWL); cols S:640 are garbage and
                        # only feed PSUM rows >= ksz, which are never read
                        kw = 640 if nm == "k" else S
                        t_ = qkpool.tile([HD, kw], FP16, tag=nm)
                        nc.sync.dma_start(t_[:, :S], src[r0:r0 + HD, tb:tb + S])
                        tsw = qkpool.tile([HD, S], FP16, tag=nm + "s")
                        tsw_r = tsw[:].rearrange("(p two) t -> p two t", two=2)
                        # issue on ACT queue: SP.SEQ is the DMA-issue bottleneck
                        nc.scalar.dma_start(tsw_r[:, 0, :], swsrc[hp:hp + 44, 1, tb:tb + S])
                        nc.scalar.dma_start(tsw_r[:, 1, :], swsrc[hp:hp + 44, 0, tb:tb + S])
                        nc.vector.tensor_mul(t_[:, :S], t_[:, :S], cos_sb[:, :S])
                        nc.vector.tensor_mul(tsw[:], tsw[:], sinp_sb[:, :S])
                        nc.vector.tensor_add(t_[:, :S], t_[:, :S], tsw[:])
                        tiles[nm] = t_
                    qh, kh = tiles["q"], tiles["k"]
                    nc.vector.memset(kh[:, S:640], 0.0)

                    es = []
                    for (k0, ksz) in KTS:
                        e_ = epool.tile([128, S], FP16, tag="es")
                        for (c0, cw) in QCH:
                            ps = psS.tile([128, 290], F32, tag="psS")
                            nc.tensor.matmul(
                                ps[:, :cw], kh[:, k0:k0 + 128],
                                qh[:, c0:c0 + cw], start=True, stop=True,
                            )
                            nc.scalar.activation(
                                e_[:ksz, c0:c0 + cw], ps[:ksz, :cw],
                                mybir.ActivationFunctionType.Exp, scale=SCALE,
                            )
                        es.append(e_)

                    vts = []
                    for (k0, ksz) in KTS:
                        # 128-wide stationary (not 97): FWL only triggers for
                        # exactly-128-column weight loads on real HW
                        vt = vtpool.tile([128, 128], FP16, tag="vt")
                        nc.sync.dma_start(
                            vt[:ksz, :HD], vs_d[img, k0:k0 + ksz, r0:r0 + HD])
                        nc.vector.memset(vt[:ksz, HD:96], 0.0)
                        nc.vector.memset(vt[:ksz, 96:97], 1.0)
                        nc.vector.memset(vt[:ksz, 97:128], 0.0)
                        vts.append(vt)

                    for (c0, cw) in QCH:
                        # rows 97:128 of pc are never read
                        pc = psC.tile([128, 290], F32, tag="psC")
                        for j, (k0, ksz) in enumerate(KTS):
                            nc.tensor.matmul(
                                pc[:, :cw], vts[j][:ksz, :],
                                es[j][:ksz, c0:c0 + cw],
                                start=(j == 0), stop=(j == len(KTS) - 1),
                            )
                        inv = ipool.tile([1, 290], FP16, tag="inv")
                        nc.vector.reciprocal(inv[:, :cw], pc[96:97, :cw])
                        pb = psB.tile([128, 290], F32, tag="psB")
                        nc.tensor.matmul(pb[:, :cw], ones1[:], inv[:, :cw],
                                         start=True, stop=True)
                        bb = bpool.tile([HD, 290], F32, tag="bb")
                        nc.scalar.copy(bb[:, :cw], pb[:HD, :cw])
                        stg = ctxpool.tile([HD, 290], FP16, tag="ctx")
                        nc.vector.tensor_mul(stg[:, :cw], pc[:HD, :cw], bb[:, :cw])
                        sw = min(cw, S - c0)
                        nc.sync.dma_start(
                            cs_d[img, r0:r0 + HD, c0:c0 + sw], stg[:, :sw])

            # ---------------- Phase D: O projection -------------------------
            wopool = ctx.enter_context(tc.tile_pool(name="wo", bufs=1))
            c2pool = ctx.enter_context(tc.tile_pool(name="ct2", bufs=13))
            opool = ctx.enter_context(tc.tile_pool(name="outp", bufs=3))
            # O-bias as a PE-broadcast tile added via the DVE copy (the
            # chains skip the onespad bias matmul)
            brow_o = cpool.tile([1, D], FP16)
            nc.sync.dma_start(brow_o[:], wfull_d[3, D:D + 1, :])
            obias = cpool.tile([128, D], F32)
            for (c0, cw) in VCH:
                pb = psB.tile([128, 512], F32, tag="psB")
                nc.tensor.matmul(pb[:, :cw], ones1[:], brow_o[:, c0:c0 + cw],
                                 start=True, stop=True)
                nc.scalar.copy(obias[:, c0:c0 + cw], pb[:, :cw])
            wos = []
            for (c0, cw) in VCH:
                wo_t = wopool.tile([128, NKT - 1, cw], FP16, tag=f"wo{c0}")
                w_r = wfull_d[3, :D, c0:c0 + cw].rearrange("(kt p) d -> p kt d", p=128)
                nc.gpsimd.dma_start(wo_t[:], w_r)
                wos.append(wo_t)
            for img in range(BPC):
                # batched context load: 11 wide DMAs per image on the Pool
                # queue instead of 55 x [128,128] on SP
                cts = []
                for kt in range(11):
                    ct = c2pool.tile([128, CPAD], FP16, tag="ct2")
                    nc.sync.dma_start(ct[:], cs_d[img, kt * 128:(kt + 1) * 128, :])
                    cts.append(ct)
                for tt in range(5):
                    rows = 65 if tt == 4 else 128
                    for ci, (c0, cw) in enumerate(VCH):
                        ps = psA.tile([128, 512], F32, tag="psA")
                        for kt in range(11):
                            nc.tensor.matmul(
                                ps[:, :cw],
                                cts[kt][:, tt * 128:(tt + 1) * 128],
                                wos[ci][:, kt, :],
                                start=(kt == 0), stop=(kt == 10),
                            )
                        ot = opool.tile([128, 512], FP16, tag="outp")
                        nc.vector.tensor_add(ot[:rows, :cw], ps[:rows, :cw],
                                             obias[:rows, c0:c0 + cw])
                        nc.sync.dma_start(
                            out_d[img * S + tt * 128: img * S + tt * 128 + rows,
                                  c0:c0 + cw],
                            ot[:rows, :cw],
                        )

    nc.compile()
    return nc


def _setup():
    nc = _build()
    install_neuronx_cc_hook()

    partition_name = nc.partition_id_tensor.name if nc.partition_id_tensor else None
    in_names, out_names, out_avals = [], [], []
    for alloc in nc.m.functions[0].allocations:
        if not isinstance(alloc, mybir.MemoryLocationSet):
            continue
        name = alloc.memorylocations[0].name
        if alloc.kind == "ExternalInput":
            if name != partition_name:
                in_names.append(name)
        elif alloc.kind == "ExternalOutput":
            out_names.append(name)
            out_avals.append(jax.core.ShapedArray(
                tuple(alloc.tensor_shape), mybir.dt.np(alloc.dtype)))
    n_params = len(in_names)
    n_outs = len(out_avals)
    in_names_all = in_names + out_names
    if partition_name is not None:
        in_names_all.append(partition_name)

    def _body(*args):
        operands = list(args)
        if partition_name is not None:
            from concourse.bass2jax import partition_id_tensor
            operands.append(partition_id_tensor())
        outs = _bass_exec_p.bind(
            *operands,
            out_avals=tuple(out_avals),
            in_names=tuple(in_names_all),
            out_names=tuple(out_names),
            lowering_input_output_aliases=(),
            sim_require_finite=True,
            sim_require_nnan=True,
            nc=nc,
        )
        return tuple(outs)

    devices = jax.devices()[:NCORES]
    mesh = Mesh(np.asarray(devices), ("core",))
    in_specs = (PartitionSpec("core"),) * (n_params + n_outs)
    out_specs = (PartitionSpec("core"),) * n_outs
    donate = tuple(range(n_params, n_params + n_outs))
    exec_fn = jax.jit(
        shard_map(_body, mesh=mesh, in_specs=in_specs, out_specs=out_specs,
                  check_rep=False),
        donate_argnums=donate, keep_unused=True)

    shard = NamedSharding(mesh, PartitionSpec("core"))
    zero_shapes = [(NCORES * a.shape[0], *a.shape[1:]) for a in out_avals]
    zero_dtypes = [a.dtype for a in out_avals]
    make_zeros = jax.jit(
        lambda: tuple(jnp.zeros(s, d) for s, d in zip(zero_shapes, zero_dtypes)),
        out_shardings=tuple([shard] * n_outs))

    return dict(nc=nc, in_names=in_names, exec_fn=exec_fn,
                make_zeros=make_zeros, shard=shard, mesh=mesh)


def _host_pack(hidden_states, wq, bq, wk, bk, wv, bv, wo, bo, cos_t, sinp_t):
    """Build the global (concatenated-over-cores) input arrays, all fp16."""
    x16 = np.asarray(hidden_states, np.float32).reshape(NCORES * TOK, D).astype(np.float16)
    wp = np.zeros((4, KP, D), np.float16)
    for i, (w, b) in enumerate(((wq, bq), (wk, bk), (wv, bv), (wo, bo))):
        wp[i, :D] = np.asarray(w, np.float32).T.astype(np.float16)
        wp[i, D] = np.asarray(b, np.float32).astype(np.float16)
    ws = wp.reshape(NCORES, WSH)
    return {
        "xin": x16,
        "ws": ws,
        "cos": np.tile(cos_t, (NCORES, 1)),
        "sinp": np.tile(sinp_t, (NCORES, 1)),
    }


def kernel(hidden_states, wq, bq, wk, bk, wv, bv, wo, bo):
    if "st" not in _CACHE:
        _CACHE["st"] = _setup()
        _CACHE["tables"] = _rope_tables()
    st = _CACHE["st"]
    cos_t, sinp_t = _CACHE["tables"]

    # donated output buffers are device-created; issue the dispatch first so
    # it rides the tunnel while the host packs/uploads the inputs
    zeros = st["make_zeros"]()
    arrs = _host_pack(hidden_states, wq, bq, wk, bk, wv, bv, wo, bo, cos_t, sinp_t)
    args = [arrs[n] for n in st["in_names"]]
    try:
        outs = st["exec_fn"](*args, *zeros)
        out = np.asarray(outs[0])
    except jax.errors.JaxRuntimeError:
        # transient tunnel/dispatch failure: retry once with fresh buffers
        zeros = st["make_zeros"]()
        outs = st["exec_fn"](*args, *zeros)
        out = np.asarray(outs[0])
    return out.astype(np.float32).reshape(B, S, D)


# revision 26
# speedup vs baseline: 2.5338x; 2.2705x over previous
"""Llama4VisionAttention on 8 Trainium2 NeuronCores.

Data-parallel over batch (32 images -> 4 per core), weights distributed by
in-NEFF AllGather so each core only uploads 1/8 of the weight pack.

Host -> device traffic per core (all fp16):
  xin  [2308, 1408]  token-major hidden states (4 images), no padding
  ws   [1, 1081344]  this core's 1/8 shard of the packed augmented weights
  cos/sinp [88, 578] rope tables
Device -> host: out [2308, 1408] fp16.

Kernel phases:
  W : stage ws to internal DRAM, AllGather -> wfull [4, 1536, 1408]
      (wq_a, wk_a, wv_a, wo_a; row 1408 = bias, rows 1409.. = 0).
  A0: PE-transpose xin into SBUF xt_sb [128, 12, TOKP] feature-major,
      k-tile 11 holds the bias ones-row.
  A : QKV projections (fp16 matmuls, fp32 PSUM accum); Q/K bias fused as
      per-partition ACT bias on the spill copy, V bias as a PE-broadcast
      tile on the DVE copy; Q/K spilled feature-major (flat [D, TOK]),
      V token-major, all fp16 DRAM scratch.
  B : per (image, head): reload Q/K head-major [88, SP] (+ pair-swapped
      copy for ROPE), ROPE on DVE, scores on PE, exp on ACT, A@V with a
      ones-column for the softmax denominator, PE ones-broadcast of
      1/rowsum, normalize, spill context feature-major.
  D : O-projection, bias added as a PE-broadcast tile on the DVE copy,
      token-major fp16 out.

Execution: cached jax.jit(shard_map(bass_exec)) over 8 cores; donated
zero output buffers are created on-device (never uploaded).
"""

import math

import numpy as np
import jax
import jax.numpy as jnp
from jax.sharding import Mesh, PartitionSpec, NamedSharding
from jax.experimental.shard_map import shard_map

import concourse.bass as bass
import concourse.tile as tile
from concourse import bacc, mybir
from concourse.bass2jax import _bass_exec_p, install_neuronx_cc_hook
from concourse.masks import make_identity

F32 = mybir.dt.float32
F32R = mybir.dt.float32r
FP16 = mybir.dt.float16

H = 16
HD = 88
D = 1408          # = H * HD
S = 577
SP = 578          # padded (even) seq
B = 32
NCORES = 8
BPC = B // NCORES  # images per core
TOK = BPC * S      # 2308
TOKP = 2376        # padded xt free dim (>= 3*577 + 512 + 128 = 2371)
KP = 1536          # padded contraction dim (12 * 128)
NKT = KP // 128
CPAD = 640         # padded ctx columns (5 * 128)
WSH = 4 * KP * D // NCORES  # weight-pack shard elems per core
SCALE = 1.0 / math.sqrt(HD)

QCH = ((0, 290), (290, 287))
QCH2 = ((0, 512), (512, 512), (1024, 512), (1536, 512), (2048, 260))
VCH = ((0, 512), (512, 512), (1024, 384))
KTS = ((0, 128), (128, 128), (256, 128), (384, 128), (512, 65))
TTS = tuple((t0, min(128, TOK - t0)) for t0 in range(0, TOK, 128))


def _rope_tables():
    IDX = 24
    theta = 10000.0
    img_idx = np.arange(IDX * IDX, dtype=np.int64).reshape(-1, 1)
    img_idx = np.concatenate([img_idx, img_idx[:1]], axis=0)
    img_idx[-1, -1] = -2
    fx = (img_idx % IDX).astype(np.float64)
    fy = (img_idx // IDX).astype(np.float64)
    freq_dim = HD // 2
    rope_freq = 1.0 / (theta ** (np.arange(0, freq_dim, 2)[: freq_dim // 2].astype(np.float64) / freq_dim))
    fxf = (fx + 1)[..., None] * rope_freq[None, None, :]
    fyf = (fy + 1)[..., None] * rope_freq[None, None, :]
    freqs = np.concatenate([fxf[..., 0, :], fyf[..., 0, :]], axis=-1)  # [S,44]
    freqs = np.where(img_idx.reshape(-1, 1) < 0, 0.0, freqs)
    cos = np.cos(freqs)
    sin = np.sin(freqs)
    cos_t = np.ones((HD, SP), np.float32)
    sinp_t = np.zeros((HD, SP), np.float32)
    for hd in range(HD):
        i = hd // 2
        cos_t[hd, :S] = cos[:, i]
        sinp_t[hd, :S] = (-sin[:, i]) if hd % 2 == 0 else sin[:, i]
    return cos_t.astype(np.float16), sinp_t.astype(np.float16)


_CACHE = {}


def _build():
    nc = bacc.Bacc("TRN2", target_bir_lowering=False, debug=False, num_devices=NCORES)

    xin_d = nc.dram_tensor("xin", [TOK, D], FP16, kind="ExternalInput").ap()
    ws_d = nc.dram_tensor("ws", [1, WSH], FP16, kind="ExternalInput").ap()
    cos_d = nc.dram_tensor("cos", [HD, SP], FP16, kind="ExternalInput").ap()
    sinp_d = nc.dram_tensor("sinp", [HD, SP], FP16, kind="ExternalInput").ap()
    out_d = nc.dram_tensor("out", [TOK, D], FP16, kind="ExternalOutput").ap()

    from contextlib import ExitStack
    with tile.TileContext(nc) as tc, nc.allow_low_precision(reason="fp16 intermediates; matmuls accumulate fp32 in PSUM"):
        with ExitStack() as ctx:
            dpool = ctx.enter_context(tc.tile_pool(name="dram", bufs=1, space="DRAM"))
            cpool = ctx.enter_context(tc.tile_pool(name="const", bufs=1))
            psA = ctx.enter_context(tc.tile_pool(name="psA", bufs=3, space="PSUM"))
            psS = ctx.enter_context(tc.tile_pool(name="psS", bufs=2, space="PSUM"))
            psC = ctx.enter_context(tc.tile_pool(name="psC", bufs=2, space="PSUM"))
            psB = ctx.enter_context(tc.tile_pool(name="psB", bufs=1, space="PSUM"))
            qkpool = ctx.enter_context(tc.tile_pool(name="qk", bufs=2))
            epool = ctx.enter_context(tc.tile_pool(name="es", bufs=7))
            vtpool = ctx.enter_context(tc.tile_pool(name="vt", bufs=10))
            vsbpool = ctx.enter_context(tc.tile_pool(name="vsb", bufs=10))
            ipool = ctx.enter_context(tc.tile_pool(name="inv", bufs=2))
            bpool = ctx.enter_context(tc.tile_pool(name="bb", bufs=2))
            ctxpool = ctx.enter_context(tc.tile_pool(name="ctx", bufs=3))

            qs_t = dpool.tile([D, TOK], FP16)
            ks_t = dpool.tile([D, TOK], FP16)
            vs_t = dpool.tile([BPC, S, D], FP16)
            cs_t = dpool.tile([BPC, D, CPAD], FP16)
            wsh_t = dpool.tile([1, WSH], FP16)
            wfull_t = dpool.tile([4, KP, D], FP16)
            qs_d, ks_d, vs_d, cs_d = qs_t[:], ks_t[:], vs_t[:], cs_t[:]
            wfull_d = wfull_t[:]

            cos_sb = cpool.tile([HD, SP], FP16)
            sinp_sb = cpool.tile([HD, SP], FP16)
            nc.sync.dma_start(cos_sb[:], cos_d[:])
            nc.sync.dma_start(sinp_sb[:], sinp_d[:])
            ones1 = cpool.tile([1, 128], FP16)
            nc.gpsimd.memset(ones1[:], 1.0)
            ident = cpool.tile([128, 128], FP16)
            make_identity(nc, ident[:])

            # ---------------- Phase W: weight AllGather --------------------
            nc.sync.dma_start(wsh_t[:], ws_d[:])
            nc.gpsimd.collective_compute(
                "AllGather",
                mybir.AluOpType.bypass,
                replica_groups=[list(range(NCORES))],
                ins=[wsh_t[:]],
                outs=[wfull_t[:]],
            )

            # ---------------- Phase A (transient pools) ---------------------
            ctxA = ExitStack()
            with ctxA:
                xpool = ctxA.enter_context(tc.tile_pool(name="xt", bufs=1))
                xrpool = ctxA.enter_context(tc.tile_pool(name="xr", bufs=3))
                wpool = ctxA.enter_context(tc.tile_pool(name="wqk", bufs=2))
                vwpool = ctxA.enter_context(tc.tile_pool(name="wv", bufs=1))
                stpool = ctxA.enter_context(tc.tile_pool(name="stqk", bufs=4))

                # Phase A0: transpose xin -> xt_sb feature-major
                xt_sb = xpool.tile([128, NKT, TOKP], FP16)
                nc.vector.memset(xt_sb[:, 11, :], 0.0)
                nc.vector.memset(xt_sb[0:1, 11, :TOK], 1.0)
                for kt in range(11):
                    nc.vector.memset(xt_sb[:, kt, TOK:TOKP], 0.0)
                for (t0, rows) in TTS:
                    xr = xrpool.tile([128, D], FP16, tag="xr")
                    nc.sync.dma_start(xr[:rows, :], xin_d[t0:t0 + rows, :])
                    for ft in range(11):
                        pt = psS.tile([128, 128], FP16, tag="psS")
                        nc.tensor.transpose(
                            pt[:, :rows], xr[:rows, ft * 128:(ft + 1) * 128],
                            ident[:rows, :rows])
                        nc.scalar.copy(xt_sb[:, ft, t0:t0 + rows], pt[:, :rows])

                # [128,1] bias columns for Q/K: load the bias rows
                # contiguously, then PE-transpose 128-chunks (a scatter DMA
                # per column would cost ~10us of descriptor issue each)
                biasqk = cpool.tile([128, 2, 11], FP16)
                brow_q = cpool.tile([1, D], FP16)
                brow_k = cpool.tile([1, D], FP16)
                brows = [brow_q, brow_k]
                for w_idx in (0, 1):
                    nc.sync.dma_start(brows[w_idx][:], wfull_d[w_idx, D:D + 1, :])
                for w_idx in (0, 1):
                    for dt_ in range(11):
                        pt = psS.tile([128, 128], FP16, tag="psS")
                        nc.tensor.transpose(
                            pt[:, 0:1],
                            brows[w_idx][:, dt_ * 128:(dt_ + 1) * 128],
                            ident[:1, :1])
                        nc.scalar.copy(biasqk[:, w_idx, dt_:dt_ + 1], pt[:, 0:1])

                # Q/K projections, feature-major spill, 512-wide chains
                # across image boundaries (flat [D, TOK] scratch layout).
                # Bias is fused into the PSUM->SBUF copy (per-partition ACT
                # bias), so the chains skip the 12th (bias ones-row) k-tile.
                for w_idx, sp_d in ((0, qs_d), (1, ks_d)):
                    for dt_ in range(11):
                        wt = wpool.tile([128, NKT - 1, 128], FP16, tag="wqk")
                        w_r = wfull_d[w_idx, :D, dt_ * 128:(dt_ + 1) * 128].rearrange(
                            "(kt p) d -> p kt d", p=128)
                        nc.gpsimd.dma_start(wt[:], w_r)
                        for (c0, cw) in QCH2:
                            ps = psA.tile([128, 512], F32, tag="psA")
                            for kt in range(NKT - 1):
                                nc.tensor.matmul(
                                    ps[:, :cw],
                                    wt[:, kt, :],
                                    xt_sb[:, kt, c0:c0 + cw],
                                    start=(kt == 0), stop=(kt == NKT - 2),
                                )
                            st = stpool.tile([128, 512], FP16, tag="stv")
                            nc.scalar.activation(
                                st[:, :cw], ps[:, :cw],
                                mybir.ActivationFunctionType.Identity,
                                bias=biasqk[:, w_idx, dt_:dt_ + 1],
                            )
                            nc.scalar.dma_start(
                                sp_d[dt_ * 128:(dt_ + 1) * 128, c0:c0 + cw],
                                st[:, :cw],
                            )

                # V projection, token-major spill.  Bias varies along the
                # free (channel) dim, so it is added as a PE-broadcast tile
                # via the DVE copy -- the chains skip the ones-row k-tile.
                brow_v = cpool.tile([1, D], FP16)
                nc.sync.dma_start(brow_v[:], wfull_d[2, D:D + 1, :])
                vbias = cpool.tile([128, D], F32)
                for (c0, cw) in VCH:
                    pb = psB.tile([128, 512], F32, tag="psB")
                    nc.tensor.matmul(pb[:, :cw], ones1[:], brow_v[:, c0:c0 + cw],
                                     start=True, stop=True)
                    nc.scalar.copy(vbias[:, c0:c0 + cw], pb[:, :cw])
                for (c0, cw) in VCH:
                    vw = vwpool.tile([128, NKT - 1, 512], FP16, tag="wv")
                    w_r = wfull_d[2, :D, c0:c0 + cw].rearrange("(kt p) d -> p kt d", p=128)
                    nc.gpsimd.dma_start(vw[:, :, :cw], w_r)
                    for img in range(BPC):
                        for tt in range(5):
                            t0 = img * S + tt * 128
                            ps = psA.tile([128, 512], F32, tag="psA")
                            for kt in range(NKT - 1):
                                nc.tensor.matmul(
                                    ps[:, :cw],
                                    xt_sb[:, kt, t0:t0 + 128],
                                    vw[:, kt, :cw],
                                    start=(kt == 0), stop=(kt == NKT - 2),
                                )
                            rows = 65 if tt == 4 else 128
                            st = stpool.tile([128, 512], FP16, tag="stv")
                            nc.vector.tensor_add(st[:rows, :cw], ps[:rows, :cw],
                                                 vbias[:rows, c0:c0 + cw])
                            nc.sync.dma_start(
                                vs_d[img, tt * 128:tt * 128 + rows, c0:c0 + cw],
                                st[:rows, :cw],
                            )

            # ---------------- Phase B: attention ---------------------------
            qs_sw = qs_d.rearrange("(p two) t -> p two t", two=2)
            ks_sw = ks_d.rearrange("(p two) t -> p two t", two=2)
            for img in range(BPC):
                tb = img * S
                # V for this image, loaded once (5 wide DMAs instead of 80
                # per-head strided loads); per-head slices are SBUF copies
                vsb = []
                for (k0, ksz) in KTS:
                    vrow = vsbpool.tile([128, D], FP16, tag="vsb")
                    nc.sync.dma_start(vrow[:ksz, :], vs_d[img, k0:k0 + ksz, :])
                    vsb.append(vrow)
                for h in range(H):
                    r0 = h * HD
                    hp = r0 // 2
                    tiles = {}
                    for nm, src, swsrc in (("q", qs_d, qs_sw), ("k", ks_d, ks_sw)):
                        # kh is 640 wide so every score-stationary slice is a
                        # full 128 columns (FWL); cols S:640 are garbage and
                        # only feed PSUM rows >= ksz, which are never read
                        kw = 640 if nm == "k" else S
                        t_ = qkpool.tile([HD, kw], FP16, tag=nm)
                        nc.sync.dma_start(t_[:, :S], src[r0:r0 + HD, tb:tb + S])
                        tsw = qkpool.tile([HD, S], FP16, tag=nm + "s")
                        tsw_r = tsw[:].rearrange("(p two) t -> p two t", two=2)
                        # issue on ACT queue: SP.SEQ is the DMA-issue bottleneck
                        nc.scalar.dma_start(tsw_r[:, 0, :], swsrc[hp:hp + 44, 1, tb:tb + S])
                        nc.scalar.dma_start(tsw_r[:, 1, :], swsrc[hp:hp + 44, 0, tb:tb + S])
                        nc.vector.tensor_mul(t_[:, :S], t_[:, :S], cos_sb[:, :S])
                        nc.vector.tensor_mul(tsw[:], tsw[:], sinp_sb[:, :S])
                        nc.vector.tensor_add(t_[:, :S], t_[:, :S], tsw[:])
                        tiles[nm] = t_
                    qh, kh = tiles["q"], tiles["k"]
                    nc.vector.memset(kh[:, S:640], 0.0)

                    es = []
                    for (k0, ksz) in KTS:
                        e_ = epool.tile([128, S], FP16, tag="es")
                        for (c0, cw) in QCH:
                            ps = psS.tile([128, 290], F32, tag="psS")
                            nc.tensor.matmul(
                                ps[:, :cw], kh[:, k0:k0 + 128],
                                qh[:, c0:c0 + cw], start=True, stop=True,
                            )
                            nc.scalar.activation(
                                e_[:ksz, c0:c0 + cw], ps[:ksz, :cw],
                                mybir.ActivationFunctionType.Exp, scale=SCALE,
                            )
                        es.append(e_)

                    vts = []
                    for j, (k0, ksz) in enumerate(KTS):
                        # 128-wide stationary (not 97): FWL only triggers for
                        # exactly-128-column weight loads on real HW
                        vt = vtpool.tile([128, 128], FP16, tag="vt")
                        nc.gpsimd.tensor_copy(vt[:ksz, :HD],
                                              vsb[j][:ksz, r0:r0 + HD])
                        nc.vector.memset(vt[:ksz, HD:96], 0.0)
                        nc.vector.memset(vt[:ksz, 96:97], 1.0)
                        nc.vector.memset(vt[:ksz, 97:128], 0.0)
                        vts.append(vt)

                    for (c0, cw) in QCH:
                        # rows 97:128 of pc are never read
                        pc = psC.tile([128, 290], F32, tag="psC")
                        for j, (k0, ksz) in enumerate(KTS):
                            nc.tensor.matmul(
                                pc[:, :cw], vts[j][:ksz, :],
                                es[j][:ksz, c0:c0 + cw],
                                start=(j == 0), stop=(j == len(KTS) - 1),
                            )
                        inv = ipool.tile([1, 290], FP16, tag="inv")
                        nc.vector.reciprocal(inv[:, :cw], pc[96:97, :cw])
                        pb = psB.tile([128, 290], F32, tag="psB")
                        nc.tensor.matmul(pb[:, :cw], ones1[:], inv[:, :cw],
                                         start=True, stop=True)
                        bb = bpool.tile([HD, 290], F32, tag="bb")
                        nc.scalar.copy(bb[:, :cw], pb[:HD, :cw])
                        stg = ctxpool.tile([HD, 290], FP16, tag="ctx")
                        nc.vector.tensor_mul(stg[:, :cw], pc[:HD, :cw], bb[:, :cw])
                        sw = min(cw, S - c0)
                        nc.sync.dma_start(
                            cs_d[img, r0:r0 + HD, c0:c0 + sw], stg[:, :sw])

            # ---------------- Phase D: O projection -------------------------
            wopool = ctx.enter_context(tc.tile_pool(name="wo", bufs=1))
            c2pool = ctx.enter_context(tc.tile_pool(name="ct2", bufs=13))
            opool = ctx.enter_context(tc.tile_pool(name="outp", bufs=3))
            # O-bias as a PE-broadcast tile added via the DVE copy (the
            # chains skip the onespad bias matmul)
            brow_o = cpool.tile([1, D], FP16)
            nc.sync.dma_start(brow_o[:], wfull_d[3, D:D + 1, :])
            obias = cpool.tile([128, D], F32)
            for (c0, cw) in VCH:
                pb = psB.tile([128, 512], F32, tag="psB")
                nc.tensor.matmul(pb[:, :cw], ones1[:], brow_o[:, c0:c0 + cw],
                                 start=True, stop=True)
                nc.scalar.copy(obias[:, c0:c0 + cw], pb[:, :cw])
            wos = []
            for (c0, cw) in VCH:
                wo_t = wopool.tile([128, NKT - 1, cw], FP16, tag=f"wo{c0}")
                w_r = wfull_d[3, :D, c0:c0 + cw].rearrange("(kt p) d -> p kt d", p=128)
                nc.gpsimd.dma_start(wo_t[:], w_r)
                wos.append(wo_t)
            for img in range(BPC):
                # batched context load: 11 wide DMAs per image on the Pool
                # queue instead of 55 x [128,128] on SP
                cts = []
                for kt in range(11):
                    ct = c2pool.tile([128, CPAD], FP16, tag="ct2")
                    nc.sync.dma_start(ct[:], cs_d[img, kt * 128:(kt + 1) * 128, :])
                    cts.append(ct)
                for tt in range(5):
                    rows = 65 if tt == 4 else 128
                    for ci, (c0, cw) in enumerate(VCH):
                        ps = psA.tile([128, 512], F32, tag="psA")
                        for kt in range(11):
                            nc.tensor.matmul(
                                ps[:, :cw],
                                cts[kt][:, tt * 128:(tt + 1) * 128],
                                wos[ci][:, kt, :],
                                start=(kt == 0), stop=(kt == 10),
                            )
                        ot = opool.tile([128, 512], FP16, tag="outp")
                        nc.vector.tensor_add(ot[:rows, :cw], ps[:rows, :cw],
                                             obias[:rows, c0:c0 + cw])
                        nc.sync.dma_start(
                            out_d[img * S + tt * 128: img * S + tt * 128 + rows,
                                  c0:c0 + cw],
                            ot[:rows, :cw],
                        )

    nc.compile()
    return nc


def _setup():
    nc = _build()
    install_neuronx_cc_hook()

    partition_name = nc.partition_id_tensor.name if nc.partition_id_tensor else None
    in_names, out_names, out_avals = [], [], []
    for alloc in nc.m.functions[0].allocations:
        if not isinstance(alloc, mybir.MemoryLocationSet):
            continue
        name = alloc.memorylocations[0].name
        if alloc.kind == "ExternalInput":
            if name != partition_name:
                in_names.append(name)
        elif alloc.kind == "ExternalOutput":
            out_names.append(name)
            out_avals.append(jax.core.ShapedArray(
                tuple(alloc.tensor_shape), mybir.dt.np(alloc.dtype)))
    n_params = len(in_names)
    n_outs = len(out_avals)
    in_names_all = in_names + out_names
    if partition_name is not None:
        in_names_all.append(partition_name)

    def _body(*args):
        operands = list(args)
        if partition_name is not None:
            from concourse.bass2jax import partition_id_tensor
            operands.append(partition_id_tensor())
        outs = _bass_exec_p.bind(
            *operands,
            out_avals=tuple(out_avals),
            in_names=tuple(in_names_all),
            out_names=tuple(out_names),
            lowering_input_output_aliases=(),
            sim_require_finite=True,
            sim_require_nnan=True,
            nc=nc,
        )
        return tuple(outs)

    devices = jax.devices()[:NCORES]
    mesh = Mesh(np.asarray(devices), ("core",))
    in_specs = (PartitionSpec("core"),) * (n_params + n_outs)
    out_specs = (PartitionSpec("core"),) * n_outs
    donate = tuple(range(n_params, n_params + n_outs))
    exec_fn = jax.jit(
        shard_map(_body, mesh=mesh, in_specs=in_specs, out_specs=out_specs,
                  check_rep=False),
        donate_argnums=donate, keep_unused=True)

    shard = NamedSharding(mesh, PartitionSpec("core"))
    zero_shapes = [(NCORES * a.shape[0], *a.shape[1:]) for a in out_avals]
    zero_dtypes = [a.dtype for a in out_avals]
    make_zeros = jax.jit(
        lambda: tuple(jnp.zeros(s, d) for s, d in zip(zero_shapes, zero_dtypes)),
        out_shardings=tuple([shard] * n_outs))

    return dict(nc=nc, in_names=in_names, exec_fn=exec_fn,
                make_zeros=make_zeros, shard=shard, mesh=mesh)


def _host_pack(hidden_states, wq, bq, wk, bk, wv, bv, wo, bo, cos_t, sinp_t):
    """Build the global (concatenated-over-cores) input arrays, all fp16."""
    x16 = np.asarray(hidden_states, np.float32).reshape(NCORES * TOK, D).astype(np.float16)
    wp = np.zeros((4, KP, D), np.float16)
    for i, (w, b) in enumerate(((wq, bq), (wk, bk), (wv, bv), (wo, bo))):
        wp[i, :D] = np.asarray(w, np.float32).T.astype(np.float16)
        wp[i, D] = np.asarray(b, np.float32).astype(np.float16)
    ws = wp.reshape(NCORES, WSH)
    return {
        "xin": x16,
        "ws": ws,
        "cos": np.tile(cos_t, (NCORES, 1)),
        "sinp": np.tile(sinp_t, (NCORES, 1)),
    }


def kernel(hidden_states, wq, bq, wk, bk, wv, bv, wo, bo):
    if "st" not in _CACHE:
        _CACHE["st"] = _setup()
        _CACHE["tables"] = _rope_tables()
    st = _CACHE["st"]
    cos_t, sinp_t = _CACHE["tables"]

    # donated output buffers are device-created; issue the dispatch first so
    # it rides the tunnel while the host packs/uploads the inputs
    zeros = st["make_zeros"]()
    arrs = _host_pack(hidden_states, wq, bq, wk, bk, wv, bv, wo, bo, cos_t, sinp_t)
    args = [arrs[n] for n in st["in_names"]]
    try:
        outs = st["exec_fn"](*args, *zeros)
        out = np.asarray(outs[0])
    except jax.errors.JaxRuntimeError:
        # transient tunnel/dispatch failure: retry once with fresh buffers
        zeros = st["make_zeros"]()
        outs = st["exec_fn"](*args, *zeros)
        out = np.asarray(outs[0])
    return out.astype(np.float32).reshape(B, S, D)
